# revision 11
# baseline (speedup 1.0000x reference)
"""Trainium2 Bass kernel for nn_DiffusionNet (4-layer MLP 288-1024-1024-1024-7
+ Euler-angle rotation / eigenvalue postprocessing).

Strategy: pure data parallel over 8 NeuronCores (16384 batch rows each).
MLP runs in transposed-activation layout (features on partitions, rows in the
free dim) with fp32r matmuls (full PE rate, ~1.8e-4 rel err), weights resident
in SBUF. Layer 4 produces [rows, 7] chunks whose values feed an on-chip
elementwise postprocessing stage (sigmoid/tanh/sin with range-reduction
identities) that assembles the full 28-column output row (R 9, E 9, D 9, S0 1).
Output leaves the device in raw SBUF layout [128, 128*28] and is untangled on
the host.
"""
import numpy as np

import concourse.bass as bass
import concourse.mybir as mybir
import concourse.tile as tile
from concourse import bacc, bass_utils

AFT = mybir.ActivationFunctionType
ALU = mybir.AluOpType
F32 = mybir.dt.float32
F32R = mybir.dt.float32r

THRESHOLD_EIGVAL = 0.003
SIZES = [288, 1024, 1024, 1024, 7]
BATCH = 131072
NCORES = 8
BCORE = BATCH // NCORES          # 16384 rows per core
NR = 512                         # rows per block (one PSUM bank width)
NB = BCORE // NR                 # 32 blocks
NCHUNK = BCORE // 128            # 128 row-chunks of 128 (for L4 / postproc)
PI = float(np.pi)

# K-chunking of the contraction dims
K1 = [(0, 128), (128, 128), (256, 32)]           # 288
K8 = [(k * 128, 128) for k in range(8)]          # 1024


def _build(repeat=1):
    nc = bacc.Bacc("TRN2", target_bir_lowering=False, debug=False,
                   num_devices=NCORES)

    # Const APs for ACT bias values used by the trig identities.
    def register_const(value):
        t = nc.alloc_sbuf_tensor(f"cst-{value}", [128, 1], F32)
        nc.gpsimd.memset(t.ap(), value)
        nc.const_aps.aps[(F32, value)] = t.ap()

    register_const(-PI / 2)
    register_const(PI / 2)
    nc.all_engine_barrier()

    xt_d = nc.dram_tensor("xt", [288, BCORE], F32, kind="ExternalInput")
    w1_d = nc.dram_tensor("w1", [288, 1024], F32, kind="ExternalInput")
    w2_d = nc.dram_tensor("w2", [1024, 1024], F32, kind="ExternalInput")
    w3_d = nc.dram_tensor("w3", [1024, 1024], F32, kind="ExternalInput")
    w4r_d = nc.dram_tensor("w4r", [128, 64], F32, kind="ExternalInput")
    b1r_d = nc.dram_tensor("b1r", [128, 8], F32, kind="ExternalInput")
    b2r_d = nc.dram_tensor("b2r", [128, 8], F32, kind="ExternalInput")
    b3r_d = nc.dram_tensor("b3r", [128, 8], F32, kind="ExternalInput")
    b4r_d = nc.dram_tensor("b4r", [128, 7], F32, kind="ExternalInput")
    out_d = nc.dram_tensor("out_sb", [128, NCHUNK * 28], F32,
                           kind="ExternalOutput")

    with tile.TileContext(nc) as tc:
        with tc.tile_pool(name="w", bufs=1) as wpool, \
             tc.tile_pool(name="stage", bufs=1) as stage, \
             tc.tile_pool(name="x", bufs=2) as xpool, \
             tc.tile_pool(name="xq", bufs=1) as xqpool, \
             tc.tile_pool(name="h", bufs=1) as hpool, \
             tc.tile_pool(name="pp", bufs=1) as pp, \
             tc.tile_pool(name="ps", bufs=8, space="PSUM") as pspool:

            # ---- load + fp32r-round the weights (resident for the whole run)
            def load_wr(dram, kchunks, name):
                tiles = []
                for i, (ofs, ksz) in enumerate(kchunks):
                    s = stage.tile([128, 1024], F32, tag="wstage")
                    nc.gpsimd.dma_start(s[:ksz, :], dram[ofs:ofs + ksz, :])
                    t = wpool.tile([ksz, 1024], F32R, tag=f"{name}_{i}")
                    nc.scalar.activation(t[:], s[:ksz, :], AFT.Copy)
                    tiles.append(t)
                return tiles

            w1r = load_wr(w1_d, K1, "w1")
            w2r = load_wr(w2_d, K8, "w2")
            w3r = load_wr(w3_d, K8, "w3")

            s = stage.tile([128, 1024], F32, tag="wstage")
            nc.gpsimd.dma_start(s[:, :64], w4r_d[:, :])
            w4t = wpool.tile([128, 64], F32R, tag="w4")
            nc.scalar.activation(w4t[:], s[:, :64], AFT.Copy)

            def load_plain(dram, cols, name):
                t = wpool.tile([128, cols], F32, tag=name)
                nc.gpsimd.dma_start(t[:], dram[:, :])
                return t

            b1t = load_plain(b1r_d, 8, "b1")
            b2t = load_plain(b2r_d, 8, "b2")
            b3t = load_plain(b3r_d, 8, "b3")
            b4t = load_plain(b4r_d, 7, "b4")

            # L4 results, value-minor per 128-row chunk: V2[p, t*7+k]
            V2 = wpool.tile([128, NCHUNK * 7], F32, tag="v2")
            OUT = wpool.tile([128, NCHUNK * 28], F32, tag="out")

            # ---- MLP over row blocks
            for blk in [b for _ in range(repeat) for b in range(NB)]:
                c0 = blk * NR
                xr = []
                for i, (ofs, ksz) in enumerate(K1):
                    xs = xpool.tile([ksz, NR], F32, tag=f"xs{i}")
                    nc.gpsimd.dma_start(xs[:], xt_d[ofs:ofs + ksz, c0:c0 + NR])
                    xq = xqpool.tile([ksz, NR], F32R, tag=f"xr{i}")
                    nc.scalar.activation(xq[:], xs[:], AFT.Copy)
                    xr.append(xq)

                def layer(rhs_tiles, w_tiles, kchunks, bias, hname):
                    hout = []
                    for m in range(8):
                        ps = pspool.tile([128, NR], F32, tag="ps")
                        nk = len(kchunks)
                        for k in range(nk):
                            nc.tensor.matmul(
                                ps[:], w_tiles[k][:, m * 128:(m + 1) * 128],
                                rhs_tiles[k][:],
                                start=(k == 0), stop=(k == nk - 1))
                        ht = hpool.tile([128, NR], F32R, tag=f"{hname}_{m}")
                        nc.scalar.activation(ht[:], ps[:], AFT.Relu,
                                             bias=bias[:, m:m + 1])
                        hout.append(ht)
                    return hout

                h1 = layer(xr, w1r, K1, b1t, "h1")
                h2 = layer(h1, w2r, K8, b2t, "h2")
                h3 = layer(h2, w3r, K8, b3t, "h3")

                # L4: out[rows, 7] per 128-row chunk (h3 chunk stationary)
                for t in range(NR // 128):
                    ps4 = pspool.tile([128, 8], F32, tag="ps")
                    for k in range(8):
                        nc.tensor.matmul(
                            ps4[:], h3[k][:, t * 128:(t + 1) * 128],
                            w4t[:, k * 8:(k + 1) * 8],
                            start=(k == 0), stop=(k == 7))
                    tchunk = blk * (NR // 128) + t
                    nc.scalar.activation(
                        V2[:, tchunk * 7:(tchunk + 1) * 7], ps4[:, :7], AFT.Copy)

            # ---- postprocessing on V2 -> OUT
            v2v = V2[:].rearrange("p (t k) -> p k t", k=7)
            outv = OUT[:].rearrange("p (t c) -> p c t", c=28)

            def OC(c):
                return outv[:, c, :]

            def ptile(name):
                return pp.tile([128, NCHUNK], F32, tag=name, name=name)

            # d_k = sigmoid(out4_k + b4_k), s0t = tanh(out4_6 + b4_6)
            d = []
            for k in range(6):
                dk = ptile(f"d{k}")
                nc.scalar.activation(dk[:], v2v[:, k, :], AFT.Sigmoid,
                                     bias=b4t[:, k:k + 1])
                d.append(dk)
            s0t = ptile("s0t")
            nc.scalar.activation(s0t[:], v2v[:, 6, :], AFT.Tanh,
                                 bias=b4t[:, 6:7])
            nc.vector.tensor_scalar_add(OC(27), s0t[:], 1.0)

            # trig via identities (ACT Sin valid on [-pi, pi]):
            #   v = sin(pi d), u = sin(pi d - pi/2) = -cos(pi d)
            #   sin(2 pi d) = -2 u v ; cos(2 pi d) = 1 - 2 v^2
            def full_turn(dk, tagp):
                v = ptile(f"{tagp}v")
                nc.scalar.activation(v[:], dk[:], AFT.Sin, scale=PI)
                u = ptile(f"{tagp}u")
                nc.scalar.activation(u[:], dk[:], AFT.Sin, scale=PI,
                                     bias=-PI / 2)
                s = ptile(f"{tagp}s")
                nc.vector.scalar_tensor_tensor(s[:], u[:], -2.0, v[:],
                                               ALU.mult, ALU.mult)
                vsq = ptile(f"{tagp}vsq")
                nc.vector.tensor_mul(vsq[:], v[:], v[:])
                c = ptile(f"{tagp}c")
                nc.vector.tensor_scalar(c[:], vsq[:], -2.0, 1.0,
                                        ALU.mult, ALU.add)
                return s, c

            sx, cx = full_turn(d[0], "x")
            sz, cz = full_turn(d[2], "z")
            sy = ptile("sy")
            nc.scalar.activation(sy[:], d[1][:], AFT.Sin, scale=PI)
            cy = ptile("cy")
            nc.scalar.activation(cy[:], d[1][:], AFT.Sin, scale=-PI,
                                 bias=PI / 2)

            # eigenvalues -> E diagonal (cols 9, 13, 17); off-diag zeros
            nc.vector.tensor_scalar_mul(OC(9), d[3][:], THRESHOLD_EIGVAL)
            nc.vector.tensor_mul(OC(13), OC(9), d[4][:])
            nc.vector.tensor_mul(OC(17), OC(13), d[5][:])
            for c in (10, 11, 12, 14, 15, 16):
                nc.vector.memset(OC(c), 0.0)

            # R = Rz(az) Ry(ay) Rx(ax), written to cols 0..8
            # R00=cz*cy           R01=cz*sy*sx - sz*cx   R02=cz*sy*cx + sz*sx
            # R10=sz*cy           R11=sz*sy*sx + cz*cx   R12=sz*sy*cx - cz*sx
            # R20=-sy             R21=cy*sx              R22=cy*cx
            t1 = ptile("t1")   # cz*sy
            t2 = ptile("t2")   # sz*sy
            m1 = ptile("m1")
            m2 = ptile("m2")
            nc.vector.tensor_mul(OC(0), cz[:], cy[:])
            nc.vector.tensor_mul(OC(3), sz[:], cy[:])
            nc.vector.tensor_scalar_mul(OC(6), sy[:], -1.0)
            nc.vector.tensor_mul(t1[:], cz[:], sy[:])
            nc.vector.tensor_mul(t2[:], sz[:], sy[:])
            nc.vector.tensor_mul(m1[:], t1[:], sx[:])
            nc.vector.tensor_mul(m2[:], sz[:], cx[:])
            nc.vector.tensor_sub(OC(1), m1[:], m2[:])
            nc.vector.tensor_mul(m1[:], t1[:], cx[:])
            nc.vector.tensor_mul(m2[:], sz[:], sx[:])
            nc.vector.tensor_add(OC(2), m1[:], m2[:])
            nc.vector.tensor_mul(m1[:], t2[:], sx[:])
            nc.vector.tensor_mul(m2[:], cz[:], cx[:])
            nc.vector.tensor_add(OC(4), m1[:], m2[:])
            nc.vector.tensor_mul(m1[:], t2[:], cx[:])
            nc.vector.tensor_mul(m2[:], cz[:], sx[:])
            nc.vector.tensor_sub(OC(5), m1[:], m2[:])
            nc.vector.tensor_mul(OC(7), cy[:], sx[:])
            nc.vector.tensor_mul(OC(8), cy[:], cx[:])

            # D = R diag(e) R^T, cols 18..26. Q_ij = e_j * R_ij
            q = {}
            for i in range(3):
                for j in range(3):
                    qt = ptile(f"q{i}{j}")
                    nc.vector.tensor_mul(qt[:], OC(9 + 4 * j), OC(3 * i + j))
                    q[(i, j)] = qt

            def dentry(i, l, col):
                nc.vector.tensor_mul(m1[:], q[(i, 0)][:], OC(3 * l + 0))
                nc.vector.tensor_mul(m2[:], q[(i, 1)][:], OC(3 * l + 1))
                nc.vector.tensor_add(m1[:], m1[:], m2[:])
                nc.vector.tensor_mul(m2[:], q[(i, 2)][:], OC(3 * l + 2))
                nc.vector.tensor_add(OC(col), m1[:], m2[:])

            dentry(0, 0, 18)
            dentry(0, 1, 19)
            dentry(0, 2, 20)
            dentry(1, 1, 22)
            dentry(1, 2, 23)
            dentry(2, 2, 26)
            nc.vector.tensor_copy(OC(21), OC(19))   # D10 = D01
            nc.vector.tensor_copy(OC(24), OC(20))   # D20 = D02
            nc.vector.tensor_copy(OC(25), OC(23))   # D21 = D12

            nc.gpsimd.dma_start(out_d[:, :], OUT[:])

    nc.compile()
    return nc


def _prepare_in_maps(x, W1, b1, W2, b2, W3, b3, W4, b4):
    xT = np.ascontiguousarray(x.T)                       # [288, BATCH]
    w4r = np.zeros((128, 8, 8), np.float32)
    w4r[:, :, :7] = W4.reshape(8, 128, 7).transpose(1, 0, 2)
    w4r = np.ascontiguousarray(w4r.reshape(128, 64))
    b1r = np.ascontiguousarray(b1.reshape(8, 128).T)
    b2r = np.ascontiguousarray(b2.reshape(8, 128).T)
    b3r = np.ascontiguousarray(b3.reshape(8, 128).T)
    b4r = np.ascontiguousarray(np.tile(b4[None, :], (128, 1)))
    shared = {
        "w1": np.ascontiguousarray(W1), "w2": np.ascontiguousarray(W2),
        "w3": np.ascontiguousarray(W3), "w4r": w4r,
        "b1r": b1r, "b2r": b2r, "b3r": b3r, "b4r": b4r,
    }
    in_maps = []
    for c in range(NCORES):
        m = dict(shared)
        m["xt"] = np.ascontiguousarray(xT[:, c * BCORE:(c + 1) * BCORE])
        in_maps.append(m)
    return in_maps


def _assemble(results):
    full = np.empty((BATCH, 28), np.float32)
    for c, r in enumerate(results):
        blockc = r["out_sb"].reshape(128, NCHUNK, 28).transpose(1, 0, 2)
        full[c * BCORE:(c + 1) * BCORE] = blockc.reshape(BCORE, 28)
    R = np.ascontiguousarray(full[:, 0:9]).reshape(BATCH, 3, 3)
    E = np.ascontiguousarray(full[:, 9:18]).reshape(BATCH, 3, 3)
    D = np.ascontiguousarray(full[:, 18:27]).reshape(BATCH, 3, 3)
    S0 = np.ascontiguousarray(full[:, 27])
    return R, E, D, S0


def run(inputs, trace=False):
    nc = _build()
    in_maps = _prepare_in_maps(**inputs)
    try:
        res = bass_utils.run_bass_kernel_spmd(
            nc, in_maps, core_ids=list(range(NCORES)), trace=trace)
    except ModuleNotFoundError:
        res = bass_utils.run_bass_kernel_spmd(
            nc, in_maps, core_ids=list(range(NCORES)), trace=False)
    return _assemble(res.results), res


def kernel(**inputs):
    (R, E, D, S0), _ = run(inputs)
    return R, E, D, S0


# revision 12
# speedup vs baseline: 2.2540x; 2.2540x over previous
"""Trainium2 Bass kernel for nn_DiffusionNet (4-layer MLP 288-1024-1024-1024-7
+ Euler-angle rotation / eigenvalue postprocessing).

Strategy: pure data parallel over 8 NeuronCores (16384 batch rows each).
MLP runs in transposed-activation layout (features on partitions, rows in the
free dim) with fp32r matmuls (full PE rate, ~1.8e-4 rel err), weights resident
in SBUF. Layer 4 produces [rows, 7] chunks whose values feed an on-chip
elementwise postprocessing stage (sigmoid/tanh/sin with range-reduction
identities) that assembles the full 28-column output row (R 9, E 9, D 9, S0 1).
Output leaves the device in raw SBUF layout [128, 128*28] and is untangled on
the host.
"""
import numpy as np

import concourse.bass as bass
import concourse.mybir as mybir
import concourse.tile as tile
from concourse import bacc, bass_utils

AFT = mybir.ActivationFunctionType
ALU = mybir.AluOpType
F32 = mybir.dt.float32
F32R = mybir.dt.float32r
MM_DT = mybir.dt.float16

THRESHOLD_EIGVAL = 0.003
SIZES = [288, 1024, 1024, 1024, 7]
BATCH = 131072
NCORES = 8
BCORE = BATCH // NCORES          # 16384 rows per core
NR = 512                         # rows per block (one PSUM bank width)
NB = BCORE // NR                 # 32 blocks
NCHUNK = BCORE // 128            # 128 row-chunks of 128 (for L4 / postproc)
PI = float(np.pi)

# K-chunking of the contraction dims
K1 = [(0, 128), (128, 128), (256, 32)]           # 288
K8 = [(k * 128, 128) for k in range(8)]          # 1024


def _build(repeat=1):
    nc = bacc.Bacc("TRN2", target_bir_lowering=False, debug=False,
                   num_devices=NCORES)

    # Const APs for ACT bias values used by the trig identities.
    def register_const(value):
        t = nc.alloc_sbuf_tensor(f"cst-{value}", [128, 1], F32)
        nc.gpsimd.memset(t.ap(), value)
        nc.const_aps.aps[(F32, value)] = t.ap()

    register_const(-PI / 2)
    register_const(PI / 2)
    nc.all_engine_barrier()

    xt_d = nc.dram_tensor("xt", [288, BCORE], F32, kind="ExternalInput")
    w1_d = nc.dram_tensor("w1", [288, 1024], F32, kind="ExternalInput")
    w2_d = nc.dram_tensor("w2", [1024, 1024], F32, kind="ExternalInput")
    w3_d = nc.dram_tensor("w3", [1024, 1024], F32, kind="ExternalInput")
    w4r_d = nc.dram_tensor("w4r", [128, 64], F32, kind="ExternalInput")
    b1r_d = nc.dram_tensor("b1r", [128, 8], F32, kind="ExternalInput")
    b2r_d = nc.dram_tensor("b2r", [128, 8], F32, kind="ExternalInput")
    b3r_d = nc.dram_tensor("b3r", [128, 8], F32, kind="ExternalInput")
    b4r_d = nc.dram_tensor("b4r", [128, 7], F32, kind="ExternalInput")
    out_d = nc.dram_tensor("out_sb", [128, NCHUNK * 28], F32,
                           kind="ExternalOutput")

    with tile.TileContext(nc) as tc:
        with tc.tile_pool(name="w", bufs=1) as wpool, \
             tc.tile_pool(name="stage", bufs=1) as stage, \
             tc.tile_pool(name="x", bufs=2) as xpool, \
             tc.tile_pool(name="xq", bufs=2) as xqpool, \
             tc.tile_pool(name="h", bufs=2) as hpool, \
             tc.tile_pool(name="pp", bufs=1) as pp, \
             tc.tile_pool(name="ps", bufs=8, space="PSUM") as pspool:

            # ---- load + fp32r-round the weights (resident for the whole run)
            def load_wr(dram, kchunks, name):
                tiles = []
                for i, (ofs, ksz) in enumerate(kchunks):
                    s = stage.tile([128, 1024], F32, tag="wstage")
                    nc.gpsimd.dma_start(s[:ksz, :], dram[ofs:ofs + ksz, :])
                    t = wpool.tile([ksz, 1024], MM_DT, tag=f"{name}_{i}")
                    nc.scalar.activation(t[:], s[:ksz, :], AFT.Copy)
                    tiles.append(t)
                return tiles

            w1r = load_wr(w1_d, K1, "w1")
            w2r = load_wr(w2_d, K8, "w2")
            w3r = load_wr(w3_d, K8, "w3")

            s = stage.tile([128, 1024], F32, tag="wstage")
            nc.gpsimd.dma_start(s[:, :64], w4r_d[:, :])
            w4t = wpool.tile([128, 64], MM_DT, tag="w4")
            nc.scalar.activation(w4t[:], s[:, :64], AFT.Copy)

            def load_plain(dram, cols, name):
                t = wpool.tile([128, cols], F32, tag=name)
                nc.gpsimd.dma_start(t[:], dram[:, :])
                return t

            b1t = load_plain(b1r_d, 8, "b1")
            b2t = load_plain(b2r_d, 8, "b2")
            b3t = load_plain(b3r_d, 8, "b3")
            b4t = load_plain(b4r_d, 7, "b4")

            # L4 results, value-minor per 128-row chunk: V2[p, t*7+k]
            V2 = wpool.tile([128, NCHUNK * 7], F32, tag="v2")
            OUT = wpool.tile([128, NCHUNK * 28], F32, tag="out")

            # ---- MLP over row blocks
            for blk in [b for _ in range(repeat) for b in range(NB)]:
                c0 = blk * NR
                xr = []
                for i, (ofs, ksz) in enumerate(K1):
                    xs = xpool.tile([ksz, NR], F32, tag=f"xs{i}")
                    nc.gpsimd.dma_start(xs[:], xt_d[ofs:ofs + ksz, c0:c0 + NR])
                    xq = xqpool.tile([ksz, NR], MM_DT, tag=f"xr{i}")
                    nc.scalar.activation(xq[:], xs[:], AFT.Copy)
                    xr.append(xq)

                def layer(rhs_tiles, w_tiles, kchunks, bias, hname):
                    hout = []
                    for m in range(8):
                        ps = pspool.tile([128, NR], F32, tag="ps")
                        nk = len(kchunks)
                        for k in range(nk):
                            nc.tensor.matmul(
                                ps[:], w_tiles[k][:, m * 128:(m + 1) * 128],
                                rhs_tiles[k][:],
                                start=(k == 0), stop=(k == nk - 1))
                        ht = hpool.tile([128, NR], MM_DT, tag=f"{hname}_{m}")
                        nc.scalar.activation(ht[:], ps[:], AFT.Relu,
                                             bias=bias[:, m:m + 1])
                        hout.append(ht)
                    return hout

                h1 = layer(xr, w1r, K1, b1t, "h1")
                h2 = layer(h1, w2r, K8, b2t, "h2")
                h3 = layer(h2, w3r, K8, b3t, "h3")

                # L4: out[rows, 7] per 128-row chunk (h3 chunk stationary)
                for t in range(NR // 128):
                    ps4 = pspool.tile([128, 8], F32, tag="ps")
                    for k in range(8):
                        nc.tensor.matmul(
                            ps4[:], h3[k][:, t * 128:(t + 1) * 128],
                            w4t[:, k * 8:(k + 1) * 8],
                            start=(k == 0), stop=(k == 7))
                    tchunk = blk * (NR // 128) + t
                    nc.scalar.activation(
                        V2[:, tchunk * 7:(tchunk + 1) * 7], ps4[:, :7], AFT.Copy)

            # ---- postprocessing on V2 -> OUT
            v2v = V2[:].rearrange("p (t k) -> p k t", k=7)
            outv = OUT[:].rearrange("p (t c) -> p c t", c=28)

            def OC(c):
                return outv[:, c, :]

            def ptile(name):
                return pp.tile([128, NCHUNK], F32, tag=name, name=name)

            # d_k = sigmoid(out4_k + b4_k), s0t = tanh(out4_6 + b4_6)
            d = []
            for k in range(6):
                dk = ptile(f"d{k}")
                nc.scalar.activation(dk[:], v2v[:, k, :], AFT.Sigmoid,
                                     bias=b4t[:, k:k + 1])
                d.append(dk)
            s0t = ptile("s0t")
            nc.scalar.activation(s0t[:], v2v[:, 6, :], AFT.Tanh,
                                 bias=b4t[:, 6:7])
            nc.vector.tensor_scalar_add(OC(27), s0t[:], 1.0)

            # trig via identities (ACT Sin valid on [-pi, pi]):
            #   v = sin(pi d), u = sin(pi d - pi/2) = -cos(pi d)
            #   sin(2 pi d) = -2 u v ; cos(2 pi d) = 1 - 2 v^2
            def full_turn(dk, tagp):
                v = ptile(f"{tagp}v")
                nc.scalar.activation(v[:], dk[:], AFT.Sin, scale=PI)
                u = ptile(f"{tagp}u")
                nc.scalar.activation(u[:], dk[:], AFT.Sin, scale=PI,
                                     bias=-PI / 2)
                s = ptile(f"{tagp}s")
                nc.vector.scalar_tensor_tensor(s[:], u[:], -2.0, v[:],
                                               ALU.mult, ALU.mult)
                vsq = ptile(f"{tagp}vsq")
                nc.vector.tensor_mul(vsq[:], v[:], v[:])
                c = ptile(f"{tagp}c")
                nc.vector.tensor_scalar(c[:], vsq[:], -2.0, 1.0,
                                        ALU.mult, ALU.add)
                return s, c

            sx, cx = full_turn(d[0], "x")
            sz, cz = full_turn(d[2], "z")
            sy = ptile("sy")
            nc.scalar.activation(sy[:], d[1][:], AFT.Sin, scale=PI)
            cy = ptile("cy")
            nc.scalar.activation(cy[:], d[1][:], AFT.Sin, scale=-PI,
                                 bias=PI / 2)

            # eigenvalues -> E diagonal (cols 9, 13, 17); off-diag zeros
            nc.vector.tensor_scalar_mul(OC(9), d[3][:], THRESHOLD_EIGVAL)
            nc.vector.tensor_mul(OC(13), OC(9), d[4][:])
            nc.vector.tensor_mul(OC(17), OC(13), d[5][:])
            for c in (10, 11, 12, 14, 15, 16):
                nc.vector.memset(OC(c), 0.0)

            # R = Rz(az) Ry(ay) Rx(ax), written to cols 0..8
            # R00=cz*cy           R01=cz*sy*sx - sz*cx   R02=cz*sy*cx + sz*sx
            # R10=sz*cy           R11=sz*sy*sx + cz*cx   R12=sz*sy*cx - cz*sx
            # R20=-sy             R21=cy*sx              R22=cy*cx
            t1 = ptile("t1")   # cz*sy
            t2 = ptile("t2")   # sz*sy
            m1 = ptile("m1")
            m2 = ptile("m2")
            nc.vector.tensor_mul(OC(0), cz[:], cy[:])
            nc.vector.tensor_mul(OC(3), sz[:], cy[:])
            nc.vector.tensor_scalar_mul(OC(6), sy[:], -1.0)
            nc.vector.tensor_mul(t1[:], cz[:], sy[:])
            nc.vector.tensor_mul(t2[:], sz[:], sy[:])
            nc.vector.tensor_mul(m1[:], t1[:], sx[:])
            nc.vector.tensor_mul(m2[:], sz[:], cx[:])
            nc.vector.tensor_sub(OC(1), m1[:], m2[:])
            nc.vector.tensor_mul(m1[:], t1[:], cx[:])
            nc.vector.tensor_mul(m2[:], sz[:], sx[:])
            nc.vector.tensor_add(OC(2), m1[:], m2[:])
            nc.vector.tensor_mul(m1[:], t2[:], sx[:])
            nc.vector.tensor_mul(m2[:], cz[:], cx[:])
            nc.vector.tensor_add(OC(4), m1[:], m2[:])
            nc.vector.tensor_mul(m1[:], t2[:], cx[:])
            nc.vector.tensor_mul(m2[:], cz[:], sx[:])
            nc.vector.tensor_sub(OC(5), m1[:], m2[:])
            nc.vector.tensor_mul(OC(7), cy[:], sx[:])
            nc.vector.tensor_mul(OC(8), cy[:], cx[:])

            # D = R diag(e) R^T, cols 18..26. Q_ij = e_j * R_ij
            q = {}
            for i in range(3):
                for j in range(3):
                    qt = ptile(f"q{i}{j}")
                    nc.vector.tensor_mul(qt[:], OC(9 + 4 * j), OC(3 * i + j))
                    q[(i, j)] = qt

            def dentry(i, l, col):
                nc.vector.tensor_mul(m1[:], q[(i, 0)][:], OC(3 * l + 0))
                nc.vector.tensor_mul(m2[:], q[(i, 1)][:], OC(3 * l + 1))
                nc.vector.tensor_add(m1[:], m1[:], m2[:])
                nc.vector.tensor_mul(m2[:], q[(i, 2)][:], OC(3 * l + 2))
                nc.vector.tensor_add(OC(col), m1[:], m2[:])

            dentry(0, 0, 18)
            dentry(0, 1, 19)
            dentry(0, 2, 20)
            dentry(1, 1, 22)
            dentry(1, 2, 23)
            dentry(2, 2, 26)
            nc.vector.tensor_copy(OC(21), OC(19))   # D10 = D01
            nc.vector.tensor_copy(OC(24), OC(20))   # D20 = D02
            nc.vector.tensor_copy(OC(25), OC(23))   # D21 = D12

            nc.gpsimd.dma_start(out_d[:, :], OUT[:])

    nc.compile()
    return nc


def _prepare_in_maps(x, W1, b1, W2, b2, W3, b3, W4, b4):
    xT = np.ascontiguousarray(x.T)                       # [288, BATCH]
    w4r = np.zeros((128, 8, 8), np.float32)
    w4r[:, :, :7] = W4.reshape(8, 128, 7).transpose(1, 0, 2)
    w4r = np.ascontiguousarray(w4r.reshape(128, 64))
    b1r = np.ascontiguousarray(b1.reshape(8, 128).T)
    b2r = np.ascontiguousarray(b2.reshape(8, 128).T)
    b3r = np.ascontiguousarray(b3.reshape(8, 128).T)
    b4r = np.ascontiguousarray(np.tile(b4[None, :], (128, 1)))
    shared = {
        "w1": np.ascontiguousarray(W1), "w2": np.ascontiguousarray(W2),
        "w3": np.ascontiguousarray(W3), "w4r": w4r,
        "b1r": b1r, "b2r": b2r, "b3r": b3r, "b4r": b4r,
    }
    in_maps = []
    for c in range(NCORES):
        m = dict(shared)
        m["xt"] = np.ascontiguousarray(xT[:, c * BCORE:(c + 1) * BCORE])
        in_maps.append(m)
    return in_maps


def _assemble(results):
    full = np.empty((BATCH, 28), np.float32)
    for c, r in enumerate(results):
        blockc = r["out_sb"].reshape(128, NCHUNK, 28).transpose(1, 0, 2)
        full[c * BCORE:(c + 1) * BCORE] = blockc.reshape(BCORE, 28)
    R = np.ascontiguousarray(full[:, 0:9]).reshape(BATCH, 3, 3)
    E = np.ascontiguousarray(full[:, 9:18]).reshape(BATCH, 3, 3)
    D = np.ascontiguousarray(full[:, 18:27]).reshape(BATCH, 3, 3)
    S0 = np.ascontiguousarray(full[:, 27])
    return R, E, D, S0


def run(inputs, trace=False):
    nc = _build()
    in_maps = _prepare_in_maps(**inputs)
    try:
        res = bass_utils.run_bass_kernel_spmd(
            nc, in_maps, core_ids=list(range(NCORES)), trace=trace)
    except ModuleNotFoundError:
        res = bass_utils.run_bass_kernel_spmd(
            nc, in_maps, core_ids=list(range(NCORES)), trace=False)
    return _assemble(res.results), res


def kernel(**inputs):
    (R, E, D, S0), _ = run(inputs)
    return R, E, D, S0
